# revision 20
# baseline (speedup 1.0000x reference)
"""Trainium2 Bass kernel for the AttentionConvBlock problem.

Reference computation (per batch b of 8):
    q = relu(conv3x3(x, Wq) + bq); k = relu(conv3x3(x, Wk) + bk)
    v = conv3x3(x, Wv) + bv
    S = q @ k (per-channel [128,128] spatial matmul)
    P = softmax over flattened 16384 entries per (b, c)
    y = P @ v + x

Sharding: data-parallel over batch, one batch per NeuronCore (8 cores).

Per-core plan:
  - Conv as 9-shift matmul: input channels on partitions, zero-padded bf16
    copy of x in SBUF; for each output-channel block (128) and position chunk
    (512), accumulate 2 ic-chunks x 9 shifts = 18 matmuls into one PSUM bank.
    ScalarE evacuates with fused bias+ReLU/Identity (+bf16 cast).
  - Two padded x layouts: natural [h,w] for k/v convs (row-major position
    chunks) and transposed [w,h] for the q conv (column-major chunks), so
    both stream the matmul rhs with stride-1 inner dim (full PE rate) and q
    lands in HBM pre-transposed for the attention stage.
  - Attention per channel: S^T = matmul(lhsT=k, rhs=qT), exp on ScalarE
    (accum_out produces free-dim sums), global softmax sum via an
    all-ones matmul broadcast, P scaled by 1/Z, y = matmul(lhsT=P^T, rhs=v),
    fp32 residual add. Channels run in waves of 4 sharing PSUM banks; waves
    of block 0 are emission-interleaved into block 1's conv stream to keep
    TensorE dense.
"""
import os
import sys

sys.path.insert(0, "/opt/trn_rl_repo")
os.environ.setdefault("MYCRO_LOCAL_CACHE", "1")

import numpy as np
import ml_dtypes

B, C, H, W = 8, 256, 128, 128
HW = H * W
N_CORES = 8
PAD = 130  # H+2 rows/cols in the padded x buffer

_PROG = None


def _build_program():
    import concourse.bass as bass
    import concourse.tile as tile
    from concourse import bacc, mybir

    dt = mybir.dt
    AF = mybir.ActivationFunctionType

    nc = bacc.Bacc("TRN2", target_bir_lowering=False, debug=False)

    x_d = nc.dram_tensor("x", [C, H, W], dt.float32, kind="ExternalInput").ap()
    w_d = nc.dram_tensor("wpack", [108, 128, 128], dt.bfloat16, kind="ExternalInput").ap()
    b_d = nc.dram_tensor("bpack", [128, 6], dt.float32, kind="ExternalInput").ap()
    y_d = nc.dram_tensor("y", [C, H, W], dt.float32, kind="ExternalOutput").ap()

    with tile.TileContext(nc) as tc:
        from contextlib import ExitStack

        with ExitStack() as ctx:
            const = ctx.enter_context(tc.tile_pool(name="const", bufs=1))
            xstage = ctx.enter_context(tc.tile_pool(name="xstage", bufs=2))
            xpad_p = ctx.enter_context(tc.tile_pool(name="xpad", bufs=1))
            evac = ctx.enter_context(tc.tile_pool(name="evac", bufs=2))
            qload = ctx.enter_context(tc.tile_pool(name="qload", bufs=3))
            kload = ctx.enter_context(tc.tile_pool(name="kload", bufs=3))
            vload = ctx.enter_context(tc.tile_pool(name="vload", bufs=3))
            att = ctx.enter_context(tc.tile_pool(name="att", bufs=3))
            stat = ctx.enter_context(tc.tile_pool(name="stat", bufs=4))
            resid = ctx.enter_context(tc.tile_pool(name="resid", bufs=2))
            outp = ctx.enter_context(tc.tile_pool(name="outp", bufs=2))
            psum_c = ctx.enter_context(tc.tile_pool(name="psc", bufs=2, space="PSUM"))
            psum_s = ctx.enter_context(tc.tile_pool(name="pss", bufs=3, space="PSUM"))
            psum_y = ctx.enter_context(tc.tile_pool(name="psy", bufs=2, space="PSUM"))
            psum_z = ctx.enter_context(tc.tile_pool(name="psz", bufs=1, space="PSUM"))
            dram = ctx.enter_context(tc.tile_pool(name="dram", bufs=1, space="DRAM"))

            # ---- constants ----
            # weight order in w_d is q,k,v; the k slice is DMA'd first since
            # the k conv runs first
            w_sb = const.tile([128, 108, 128], dt.bfloat16)
            b_sb = const.tile([128, 6], dt.float32)
            nc.sync.dma_start(out=b_sb[:], in_=b_d)
            nc.sync.dma_start(
                out=w_sb[:, 36:72, :], in_=w_d[36:72].rearrange("t p f -> p t f")
            )
            ones_f32 = const.tile([128, 128], dt.float32)
            nc.vector.memset(ones_f32[:], 1.0)

            # ---- padded bf16 x, natural [h, w] and transposed [w, h] ----
            xpad, xpadT = [], []
            for icc in range(2):
                t = xpad_p.tile([128, PAD, PAD], dt.bfloat16, tag=f"xpad{icc}")
                tt = xpad_p.tile([128, PAD, PAD], dt.bfloat16, tag=f"xpadT{icc}")
                for b in (t, tt):
                    nc.vector.memset(b[:, 0, :], 0.0)
                    nc.vector.memset(b[:, PAD - 1, :], 0.0)
                    nc.vector.memset(b[:, :, 0:1], 0.0)
                    nc.vector.memset(b[:, :, PAD - 1 : PAD], 0.0)
                xpad.append(t)
                xpadT.append(tt)
            xflat = x_d.rearrange("c h w -> c (h w)")
            # DVE casts each staged chunk into the natural layout (k/v convs
            # consume it incrementally); the transposed buffer is filled by
            # ScalarE copies FROM xpad, deferred into the k-conv chunk
            # stream so neither the staging slots nor the ACT queue stall.
            castT_jobs = []
            for s in range(16):
                for icc in range(2):
                    st = xstage.tile([128, 1024], dt.float32, tag="st")
                    nc.sync.dma_start(
                        out=st[:],
                        in_=xflat[icc * 128 : (icc + 1) * 128, s * 1024 : (s + 1) * 1024],
                    )
                    nc.vector.tensor_copy(
                        out=xpad[icc][:, 1 + s * 8 : 1 + s * 8 + 8, 1 : 1 + W],
                        in_=st.rearrange("p (h w) -> p h w", w=W),
                    )

                    def castT(s=s, icc=icc):
                        nc.scalar.activation(
                            out=xpadT[icc][:, 1 : 1 + W, 1 + s * 8 : 1 + s * 8 + 8],
                            in_=xpad[icc][
                                :, 1 + s * 8 : 1 + s * 8 + 8, 1 : 1 + W
                            ].rearrange("p h w -> p w h"),
                            func=AF.Copy,
                        )

                    castT_jobs.append(castT)

            # remaining weights land while the k conv runs
            nc.sync.dma_start(
                out=w_sb[:, 72:108, :], in_=w_d[72:108].rearrange("t p f -> p t f")
            )
            nc.sync.dma_start(
                out=w_sb[:, 0:36, :], in_=w_d[0:36].rearrange("t p f -> p t f")
            )

            # ---- HBM round-trip buffers for q^T, k, v (bf16) ----
            # position-major [j, c, i] layout: the attention loads then read
            # one contiguous 2 KB run per partition (128 descriptors per
            # DMA instead of 1024), which keeps descriptor generation off
            # the critical path in the tail.
            qt_dram = dram.tile([128, C, 128], dt.bfloat16, tag="qt")
            k_dram = dram.tile([128, C, 128], dt.bfloat16, tag="kd")
            v_dram = dram.tile([128, C, 128], dt.bfloat16, tag="vd")
            cv_dram = [qt_dram, k_dram, v_dram]

            def conv_chunk(occ, cv, ch):
                colmajor = cv == 0
                ps = psum_c.tile([128, 512], dt.float32, tag="psc")
                for i, (icc, kk) in enumerate(
                    [(a, b) for a in range(2) for b in range(9)]
                ):
                    dy, dx = kk // 3, kk % 3
                    if colmajor:
                        rhs = xpadT[icc][
                            :, 4 * ch + dx : 4 * ch + dx + 4, dy : dy + H
                        ]
                    else:
                        rhs = xpad[icc][
                            :, 4 * ch + dy : 4 * ch + dy + 4, dx : dx + W
                        ]
                    nc.tensor.matmul(
                        ps[:],
                        lhsT=w_sb[:, cv * 36 + kk * 4 + icc * 2 + occ, :],
                        rhs=rhs,
                        start=(i == 0),
                        stop=(i == 17),
                    )
                ev = evac.tile([128, 512], dt.bfloat16, tag="ev")
                nc.scalar.activation(
                    out=ev[:],
                    in_=ps[:],
                    func=AF.Relu if cv < 2 else AF.Identity,
                    bias=b_sb[:, cv * 2 + occ : cv * 2 + occ + 1],
                    scale=1.0,
                )
                nc.sync.dma_start(
                    out=cv_dram[cv][
                        4 * ch : 4 * ch + 4, occ * 128 : (occ + 1) * 128, :
                    ].rearrange("j c i -> c j i"),
                    in_=ev.rearrange("c (j i) -> c j i", i=128),
                )

            # Attention runs in pairs of 4-channel waves: one batched DMA
            # set per 8 channels (halves Sync-queue descriptor-gen cost,
            # which otherwise paces the tail), stores on the idle GpSimd
            # queue. Per wave: 4 S-matmuls into one PSUM bank, ONE batched
            # exp on ScalarE, one DVE reduce for the per-channel sums, the
            # global-sum broadcast matmul, 1/Z scale of P, 4 y-matmuls,
            # fp32 residual add.
            def att_load(occ, g0):
                c0 = occ * 128 + g0
                qt8 = qload.tile([128, 8, 128], dt.bfloat16, tag="qt8")
                nc.sync.dma_start(out=qt8[:], in_=qt_dram[:, c0 : c0 + 8, :])
                k8 = kload.tile([128, 8, 128], dt.bfloat16, tag="k8")
                nc.sync.dma_start(out=k8[:], in_=k_dram[:, c0 : c0 + 8, :])
                v8 = vload.tile([128, 8, 128], dt.bfloat16, tag="v8")
                nc.sync.dma_start(out=v8[:], in_=v_dram[:, c0 : c0 + 8, :])
                xr8 = resid.tile([128, 8, 128], dt.float32, tag="xr8")
                nc.sync.dma_start(
                    out=xr8[:], in_=x_d[c0 : c0 + 8].rearrange("c h w -> h c w")
                )
                out8 = outp.tile([128, 8, 128], dt.float32, tag="out8")
                return qt8, k8, v8, xr8, out8

            def att_wave(tiles, w):
                qt8, k8, v8, xr8, out8 = tiles
                o = 4 * w
                ps_s = psum_s.tile([128, 4, 128], dt.float32, tag="pss")
                for j in range(4):
                    nc.tensor.matmul(
                        ps_s[:, j, :],
                        lhsT=k8[:, o + j, :],
                        rhs=qt8[:, o + j, :],
                        start=True,
                        stop=True,
                    )
                p4 = att.tile([128, 4, 128], dt.bfloat16, tag="p4")
                nc.scalar.activation(out=p4[:], in_=ps_s[:], func=AF.Exp)
                cs4 = stat.tile([128, 4], dt.float32, tag="cs4")
                nc.vector.reduce_sum(cs4[:], p4[:], axis=mybir.AxisListType.X)
                ps_z = psum_z.tile([128, 4], dt.float32, tag="psz")
                nc.tensor.matmul(
                    ps_z[:], lhsT=ones_f32[:], rhs=cs4[:], start=True, stop=True
                )
                rec4 = stat.tile([128, 4], dt.float32, tag="rec4")
                nc.vector.reciprocal(rec4[:], ps_z[:])
                ps_y = psum_y.tile([128, 4, 128], dt.float32, tag="psy")
                for j in range(4):
                    nc.tensor.matmul(
                        ps_y[:, j, :],
                        lhsT=p4[:, j, :],
                        rhs=v8[:, o + j, :],
                        start=True,
                        stop=True,
                    )
                # out = (P~ @ v) / Z + x in one fused DVE op per channel
                for j in range(4):
                    nc.vector.scalar_tensor_tensor(
                        out=out8[:, o + j, :],
                        in0=ps_y[:, j, :],
                        scalar=rec4[:, j : j + 1],
                        in1=xr8[:, o + j, :],
                        op0=mybir.AluOpType.mult,
                        op1=mybir.AluOpType.add,
                    )

            def att_store(occ, g0, tiles):
                c0 = occ * 128 + g0
                nc.gpsimd.dma_start(
                    out=y_d[c0 : c0 + 8].rearrange("c h w -> h c w"), in_=tiles[4][:]
                )

            def att_steps(occ):
                for g0 in range(0, 128, 8):
                    tiles = []

                    def load(g0=g0, tiles=tiles):
                        tiles.append(att_load(occ, g0))

                    def wave0(tiles=tiles):
                        att_wave(tiles[0], 0)

                    def wave1(g0=g0, tiles=tiles):
                        att_wave(tiles[0], 1)
                        att_store(occ, g0, tiles[0])

                    yield load
                    yield wave0
                    yield wave1

            # Phase A: block-0 convs (k, v first; q last so the xpadT fill
            # hides under the k/v matmuls). One deferred transposed-cast
            # per early chunk keeps ScalarE ahead of the PSUM evacuations.
            for i, (cv, ch) in enumerate(
                [(a, b) for a in (1, 2, 0) for b in range(32)]
            ):
                conv_chunk(0, cv, ch)
                if i % 2 == 0 and i // 2 < len(castT_jobs):
                    castT_jobs[i // 2]()
            # Phase B: block-1 convs with block-0 attention steps woven in
            # (one step per 2 conv chunks keeps TensorE dense while the
            # attention DMA/ACT/DVE chain rides along).
            steps = att_steps(0)
            for i, (cv, ch) in enumerate(
                [(a, b) for a in (1, 2, 0) for b in range(32)]
            ):
                conv_chunk(1, cv, ch)
                if i % 2 == 1:
                    step = next(steps, None)
                    if step is not None:
                        step()
            for step in steps:
                step()
            # Phase C: block-1 attention tail.
            for step in att_steps(1):
                step()

    nc.compile()
    return nc


def _get_program():
    global _PROG
    if _PROG is None:
        _PROG = _build_program()
    return _PROG


def _pack_weights(Wq, Wk, Wv):
    packs = []
    for Wcv in (Wq, Wk, Wv):
        a = np.asarray(Wcv, np.float32).transpose(2, 3, 1, 0)  # [dy, dx, i, o]
        a = a.reshape(9, 2, 128, 2, 128)  # [kk, icc, i, occ, o]
        a = a.transpose(0, 1, 3, 2, 4)  # [kk, icc, occ, i, o]
        packs.append(a.reshape(36, 128, 128))
    return np.concatenate(packs, axis=0).astype(ml_dtypes.bfloat16)


def _run(inputs, trace=False, trace_kwargs=None):
    from concourse.bass_utils import run_bass_kernel_spmd

    nc = _get_program()
    x = np.ascontiguousarray(np.asarray(inputs["x"], np.float32))
    wpack = _pack_weights(inputs["Wq"], inputs["Wk"], inputs["Wv"])
    bq = np.asarray(inputs["bq"], np.float32)
    bk = np.asarray(inputs["bk"], np.float32)
    bv = np.asarray(inputs["bv"], np.float32)
    bpack = np.stack(
        [bq[:128], bq[128:], bk[:128], bk[128:], bv[:128], bv[128:]], axis=1
    )
    bpack = np.ascontiguousarray(bpack, dtype=np.float32)  # [128, 6]

    in_maps = [
        {"x": x[b], "wpack": wpack, "bpack": bpack} for b in range(N_CORES)
    ]
    res = run_bass_kernel_spmd(
        nc,
        in_maps,
        core_ids=list(range(N_CORES)),
        trace=trace,
        **(trace_kwargs or {}),
    )
    out = np.stack([res.results[b]["y"] for b in range(N_CORES)], axis=0)
    return out, res


def kernel(**inputs) -> np.ndarray:
    out, _ = _run(inputs, trace=False)
    return out


def kernel_traced(inputs):
    try:
        import axon_shim

        axon_shim.install()
    except Exception:
        pass
    out, res = _run(inputs, trace=True)
    return out, res


# revision 21
# speedup vs baseline: 1.0017x; 1.0017x over previous
"""Trainium2 Bass kernel for the AttentionConvBlock problem.

Reference computation (per batch b of 8):
    q = relu(conv3x3(x, Wq) + bq); k = relu(conv3x3(x, Wk) + bk)
    v = conv3x3(x, Wv) + bv
    S = q @ k (per-channel [128,128] spatial matmul)
    P = softmax over flattened 16384 entries per (b, c)
    y = P @ v + x

Sharding: data-parallel over batch, one batch per NeuronCore (8 cores).

Per-core plan:
  - Conv as 9-shift matmul: input channels on partitions, zero-padded bf16
    copy of x in SBUF; for each output-channel block (128) and position chunk
    (512), accumulate 2 ic-chunks x 9 shifts = 18 matmuls into one PSUM bank.
    ScalarE evacuates with fused bias+ReLU/Identity (+bf16 cast).
  - Two padded x layouts: natural [h,w] for k/v convs (row-major position
    chunks) and transposed [w,h] for the q conv (column-major chunks), so
    both stream the matmul rhs with stride-1 inner dim (full PE rate) and q
    lands in HBM pre-transposed for the attention stage.
  - Attention per channel: S^T = matmul(lhsT=k, rhs=qT), exp on ScalarE
    (accum_out produces free-dim sums), global softmax sum via an
    all-ones matmul broadcast, P scaled by 1/Z, y = matmul(lhsT=P^T, rhs=v),
    fp32 residual add. Channels run in waves of 4 sharing PSUM banks; waves
    of block 0 are emission-interleaved into block 1's conv stream to keep
    TensorE dense.
"""
import os
import sys

sys.path.insert(0, "/opt/trn_rl_repo")
os.environ.setdefault("MYCRO_LOCAL_CACHE", "1")

import numpy as np
import ml_dtypes

B, C, H, W = 8, 256, 128, 128
HW = H * W
N_CORES = 8
PAD = 130  # H+2 rows/cols in the padded x buffer

_PROG = None


def _build_program():
    import concourse.bass as bass
    import concourse.tile as tile
    from concourse import bacc, mybir

    dt = mybir.dt
    AF = mybir.ActivationFunctionType

    nc = bacc.Bacc("TRN2", target_bir_lowering=False, debug=False)

    x_d = nc.dram_tensor("x", [C, H, W], dt.float32, kind="ExternalInput").ap()
    w_d = nc.dram_tensor("wpack", [108, 128, 128], dt.bfloat16, kind="ExternalInput").ap()
    b_d = nc.dram_tensor("bpack", [128, 6], dt.float32, kind="ExternalInput").ap()
    y_d = nc.dram_tensor("y", [C, H, W], dt.float32, kind="ExternalOutput").ap()

    with tile.TileContext(nc) as tc:
        from contextlib import ExitStack

        with ExitStack() as ctx:
            const = ctx.enter_context(tc.tile_pool(name="const", bufs=1))
            xstage = ctx.enter_context(tc.tile_pool(name="xstage", bufs=2))
            xpad_p = ctx.enter_context(tc.tile_pool(name="xpad", bufs=1))
            evac = ctx.enter_context(tc.tile_pool(name="evac", bufs=2))
            qload = ctx.enter_context(tc.tile_pool(name="qload", bufs=3))
            kload = ctx.enter_context(tc.tile_pool(name="kload", bufs=3))
            vload = ctx.enter_context(tc.tile_pool(name="vload", bufs=3))
            att = ctx.enter_context(tc.tile_pool(name="att", bufs=2))
            stat = ctx.enter_context(tc.tile_pool(name="stat", bufs=4))
            resid = ctx.enter_context(tc.tile_pool(name="resid", bufs=2))
            outp = ctx.enter_context(tc.tile_pool(name="outp", bufs=2))
            psum_c = ctx.enter_context(tc.tile_pool(name="psc", bufs=3, space="PSUM"))
            psum_s = ctx.enter_context(tc.tile_pool(name="pss", bufs=2, space="PSUM"))
            psum_y = ctx.enter_context(tc.tile_pool(name="psy", bufs=2, space="PSUM"))
            psum_z = ctx.enter_context(tc.tile_pool(name="psz", bufs=1, space="PSUM"))
            dram = ctx.enter_context(tc.tile_pool(name="dram", bufs=1, space="DRAM"))

            # ---- constants ----
            # weight order in w_d is q,k,v; the k slice is DMA'd first since
            # the k conv runs first
            w_sb = const.tile([128, 108, 128], dt.bfloat16)
            b_sb = const.tile([128, 6], dt.float32)
            nc.sync.dma_start(out=b_sb[:], in_=b_d)
            nc.sync.dma_start(
                out=w_sb[:, 36:72, :], in_=w_d[36:72].rearrange("t p f -> p t f")
            )
            ones_f32 = const.tile([128, 128], dt.float32)
            nc.vector.memset(ones_f32[:], 1.0)

            # ---- padded bf16 x, natural [h, w] and transposed [w, h] ----
            xpad, xpadT = [], []
            for icc in range(2):
                t = xpad_p.tile([128, PAD, PAD], dt.bfloat16, tag=f"xpad{icc}")
                tt = xpad_p.tile([128, PAD, PAD], dt.bfloat16, tag=f"xpadT{icc}")
                for b in (t, tt):
                    nc.vector.memset(b[:, 0, :], 0.0)
                    nc.vector.memset(b[:, PAD - 1, :], 0.0)
                    nc.vector.memset(b[:, :, 0:1], 0.0)
                    nc.vector.memset(b[:, :, PAD - 1 : PAD], 0.0)
                xpad.append(t)
                xpadT.append(tt)
            xflat = x_d.rearrange("c h w -> c (h w)")
            # DVE casts each staged chunk into the natural layout (k/v convs
            # consume it incrementally); the transposed buffer is filled by
            # ScalarE copies FROM xpad, deferred into the k-conv chunk
            # stream so neither the staging slots nor the ACT queue stall.
            castT_jobs = []
            for s in range(16):
                for icc in range(2):
                    st = xstage.tile([128, 1024], dt.float32, tag="st")
                    nc.sync.dma_start(
                        out=st[:],
                        in_=xflat[icc * 128 : (icc + 1) * 128, s * 1024 : (s + 1) * 1024],
                    )
                    nc.vector.tensor_copy(
                        out=xpad[icc][:, 1 + s * 8 : 1 + s * 8 + 8, 1 : 1 + W],
                        in_=st.rearrange("p (h w) -> p h w", w=W),
                    )

                    def castT(s=s, icc=icc):
                        nc.scalar.activation(
                            out=xpadT[icc][:, 1 : 1 + W, 1 + s * 8 : 1 + s * 8 + 8],
                            in_=xpad[icc][
                                :, 1 + s * 8 : 1 + s * 8 + 8, 1 : 1 + W
                            ].rearrange("p h w -> p w h"),
                            func=AF.Copy,
                        )

                    castT_jobs.append(castT)

            # remaining weights land while the k conv runs
            nc.sync.dma_start(
                out=w_sb[:, 72:108, :], in_=w_d[72:108].rearrange("t p f -> p t f")
            )
            nc.sync.dma_start(
                out=w_sb[:, 0:36, :], in_=w_d[0:36].rearrange("t p f -> p t f")
            )

            # ---- HBM round-trip buffers for q^T, k, v (bf16) ----
            # position-major [j, c, i] layout: the attention loads then read
            # one contiguous 2 KB run per partition (128 descriptors per
            # DMA instead of 1024), which keeps descriptor generation off
            # the critical path in the tail.
            qt_dram = dram.tile([128, C, 128], dt.bfloat16, tag="qt")
            k_dram = dram.tile([128, C, 128], dt.bfloat16, tag="kd")
            v_dram = dram.tile([128, C, 128], dt.bfloat16, tag="vd")
            cv_dram = [qt_dram, k_dram, v_dram]

            def conv_chunk(occ, cv, ch):
                colmajor = cv == 0
                ps = psum_c.tile([128, 512], dt.float32, tag="psc")
                for i, (icc, kk) in enumerate(
                    [(a, b) for a in range(2) for b in range(9)]
                ):
                    dy, dx = kk // 3, kk % 3
                    if colmajor:
                        rhs = xpadT[icc][
                            :, 4 * ch + dx : 4 * ch + dx + 4, dy : dy + H
                        ]
                    else:
                        rhs = xpad[icc][
                            :, 4 * ch + dy : 4 * ch + dy + 4, dx : dx + W
                        ]
                    nc.tensor.matmul(
                        ps[:],
                        lhsT=w_sb[:, cv * 36 + kk * 4 + icc * 2 + occ, :],
                        rhs=rhs,
                        start=(i == 0),
                        stop=(i == 17),
                    )
                ev = evac.tile([128, 512], dt.bfloat16, tag="ev")
                nc.scalar.activation(
                    out=ev[:],
                    in_=ps[:],
                    func=AF.Relu if cv < 2 else AF.Identity,
                    bias=b_sb[:, cv * 2 + occ : cv * 2 + occ + 1],
                    scale=1.0,
                )
                nc.sync.dma_start(
                    out=cv_dram[cv][
                        4 * ch : 4 * ch + 4, occ * 128 : (occ + 1) * 128, :
                    ].rearrange("j c i -> c j i"),
                    in_=ev.rearrange("c (j i) -> c j i", i=128),
                )

            # Attention runs in pairs of 4-channel waves: one batched DMA
            # set per 8 channels (halves Sync-queue descriptor-gen cost,
            # which otherwise paces the tail), stores on the idle GpSimd
            # queue. Per wave: 4 S-matmuls into one PSUM bank, ONE batched
            # exp on ScalarE, one DVE reduce for the per-channel sums, the
            # global-sum broadcast matmul, 1/Z scale of P, 4 y-matmuls,
            # fp32 residual add.
            def att_load(occ, g0):
                c0 = occ * 128 + g0
                qt8 = qload.tile([128, 8, 128], dt.bfloat16, tag="qt8")
                nc.sync.dma_start(out=qt8[:], in_=qt_dram[:, c0 : c0 + 8, :])
                k8 = kload.tile([128, 8, 128], dt.bfloat16, tag="k8")
                nc.sync.dma_start(out=k8[:], in_=k_dram[:, c0 : c0 + 8, :])
                v8 = vload.tile([128, 8, 128], dt.bfloat16, tag="v8")
                nc.sync.dma_start(out=v8[:], in_=v_dram[:, c0 : c0 + 8, :])
                xr8 = resid.tile([128, 8, 128], dt.float32, tag="xr8")
                nc.sync.dma_start(
                    out=xr8[:], in_=x_d[c0 : c0 + 8].rearrange("c h w -> h c w")
                )
                out8 = outp.tile([128, 8, 128], dt.float32, tag="out8")
                return qt8, k8, v8, xr8, out8

            def att_wave(tiles, w):
                qt8, k8, v8, xr8, out8 = tiles
                o = 4 * w
                ps_s = psum_s.tile([128, 4, 128], dt.float32, tag="pss")
                for j in range(4):
                    nc.tensor.matmul(
                        ps_s[:, j, :],
                        lhsT=k8[:, o + j, :],
                        rhs=qt8[:, o + j, :],
                        start=True,
                        stop=True,
                    )
                p4 = att.tile([128, 4, 128], dt.bfloat16, tag="p4")
                nc.scalar.activation(out=p4[:], in_=ps_s[:], func=AF.Exp)
                cs4 = stat.tile([128, 4], dt.float32, tag="cs4")
                nc.vector.reduce_sum(cs4[:], p4[:], axis=mybir.AxisListType.X)
                ps_z = psum_z.tile([128, 4], dt.float32, tag="psz")
                nc.tensor.matmul(
                    ps_z[:], lhsT=ones_f32[:], rhs=cs4[:], start=True, stop=True
                )
                rec4 = stat.tile([128, 4], dt.float32, tag="rec4")
                nc.vector.reciprocal(rec4[:], ps_z[:])
                ps_y = psum_y.tile([128, 4, 128], dt.float32, tag="psy")
                for j in range(4):
                    nc.tensor.matmul(
                        ps_y[:, j, :],
                        lhsT=p4[:, j, :],
                        rhs=v8[:, o + j, :],
                        start=True,
                        stop=True,
                    )
                # out = (P~ @ v) / Z + x in one fused DVE op per channel
                for j in range(4):
                    nc.vector.scalar_tensor_tensor(
                        out=out8[:, o + j, :],
                        in0=ps_y[:, j, :],
                        scalar=rec4[:, j : j + 1],
                        in1=xr8[:, o + j, :],
                        op0=mybir.AluOpType.mult,
                        op1=mybir.AluOpType.add,
                    )

            def att_store(occ, g0, tiles):
                c0 = occ * 128 + g0
                nc.gpsimd.dma_start(
                    out=y_d[c0 : c0 + 8].rearrange("c h w -> h c w"), in_=tiles[4][:]
                )

            def att_steps(occ):
                for g0 in range(0, 128, 8):
                    tiles = []

                    def load(g0=g0, tiles=tiles):
                        tiles.append(att_load(occ, g0))

                    def wave0(tiles=tiles):
                        att_wave(tiles[0], 0)

                    def wave1(g0=g0, tiles=tiles):
                        att_wave(tiles[0], 1)
                        att_store(occ, g0, tiles[0])

                    yield load
                    yield wave0
                    yield wave1

            # Phase A: block-0 convs (k, v first; q last so the xpadT fill
            # hides under the k/v matmuls). One deferred transposed-cast
            # per early chunk keeps ScalarE ahead of the PSUM evacuations.
            for i, (cv, ch) in enumerate(
                [(a, b) for a in (1, 2, 0) for b in range(32)]
            ):
                conv_chunk(0, cv, ch)
                if i % 2 == 0 and i // 2 < len(castT_jobs):
                    castT_jobs[i // 2]()
            # Phase B: block-1 convs with block-0 attention steps woven in
            # (one step per 2 conv chunks keeps TensorE dense while the
            # attention DMA/ACT/DVE chain rides along).
            steps = att_steps(0)
            for i, (cv, ch) in enumerate(
                [(a, b) for a in (1, 2, 0) for b in range(32)]
            ):
                conv_chunk(1, cv, ch)
                if i % 2 == 1:
                    step = next(steps, None)
                    if step is not None:
                        step()
            for step in steps:
                step()
            # Phase C: block-1 attention tail.
            for step in att_steps(1):
                step()

    nc.compile()
    return nc


def _get_program():
    global _PROG
    if _PROG is None:
        _PROG = _build_program()
    return _PROG


def _pack_weights(Wq, Wk, Wv):
    packs = []
    for Wcv in (Wq, Wk, Wv):
        a = np.asarray(Wcv, np.float32).transpose(2, 3, 1, 0)  # [dy, dx, i, o]
        a = a.reshape(9, 2, 128, 2, 128)  # [kk, icc, i, occ, o]
        a = a.transpose(0, 1, 3, 2, 4)  # [kk, icc, occ, i, o]
        packs.append(a.reshape(36, 128, 128))
    return np.concatenate(packs, axis=0).astype(ml_dtypes.bfloat16)


def _run(inputs, trace=False, trace_kwargs=None):
    from concourse.bass_utils import run_bass_kernel_spmd

    nc = _get_program()
    x = np.ascontiguousarray(np.asarray(inputs["x"], np.float32))
    wpack = _pack_weights(inputs["Wq"], inputs["Wk"], inputs["Wv"])
    bq = np.asarray(inputs["bq"], np.float32)
    bk = np.asarray(inputs["bk"], np.float32)
    bv = np.asarray(inputs["bv"], np.float32)
    bpack = np.stack(
        [bq[:128], bq[128:], bk[:128], bk[128:], bv[:128], bv[128:]], axis=1
    )
    bpack = np.ascontiguousarray(bpack, dtype=np.float32)  # [128, 6]

    in_maps = [
        {"x": x[b], "wpack": wpack, "bpack": bpack} for b in range(N_CORES)
    ]
    res = run_bass_kernel_spmd(
        nc,
        in_maps,
        core_ids=list(range(N_CORES)),
        trace=trace,
        **(trace_kwargs or {}),
    )
    out = np.stack([res.results[b]["y"] for b in range(N_CORES)], axis=0)
    return out, res


def kernel(**inputs) -> np.ndarray:
    out, _ = _run(inputs, trace=False)
    return out


def kernel_traced(inputs):
    try:
        import axon_shim

        axon_shim.install()
    except Exception:
        pass
    out, res = _run(inputs, trace=True)
    return out, res


# revision 22
# speedup vs baseline: 1.0090x; 1.0073x over previous
"""Trainium2 Bass kernel for the AttentionConvBlock problem.

Reference computation (per batch b of 8):
    q = relu(conv3x3(x, Wq) + bq); k = relu(conv3x3(x, Wk) + bk)
    v = conv3x3(x, Wv) + bv
    S = q @ k (per-channel [128,128] spatial matmul)
    P = softmax over flattened 16384 entries per (b, c)
    y = P @ v + x

Sharding: data-parallel over batch, one batch per NeuronCore (8 cores).

Per-core plan:
  - Conv as 9-shift matmul: input channels on partitions, zero-padded bf16
    copy of x in SBUF; for each output-channel block (128) and position chunk
    (512), accumulate 2 ic-chunks x 9 shifts = 18 matmuls into one PSUM bank.
    ScalarE evacuates with fused bias+ReLU/Identity (+bf16 cast).
  - Two padded x layouts: natural [h,w] for k/v convs (row-major position
    chunks) and transposed [w,h] for the q conv (column-major chunks), so
    both stream the matmul rhs with stride-1 inner dim (full PE rate) and q
    lands in HBM pre-transposed for the attention stage.
  - Attention per channel: S^T = matmul(lhsT=k, rhs=qT), exp on ScalarE
    (accum_out produces free-dim sums), global softmax sum via an
    all-ones matmul broadcast, P scaled by 1/Z, y = matmul(lhsT=P^T, rhs=v),
    fp32 residual add. Channels run in waves of 4 sharing PSUM banks; waves
    of block 0 are emission-interleaved into block 1's conv stream to keep
    TensorE dense.
"""
import os
import sys

sys.path.insert(0, "/opt/trn_rl_repo")
os.environ.setdefault("MYCRO_LOCAL_CACHE", "1")

import numpy as np
import ml_dtypes

B, C, H, W = 8, 256, 128, 128
HW = H * W
N_CORES = 8
PAD = 130  # H+2 rows/cols in the padded x buffer

_PROG = None


def _build_program():
    import concourse.bass as bass
    import concourse.tile as tile
    from concourse import bacc, mybir

    dt = mybir.dt
    AF = mybir.ActivationFunctionType

    nc = bacc.Bacc("TRN2", target_bir_lowering=False, debug=False)

    x_d = nc.dram_tensor("x", [C, H, W], dt.float32, kind="ExternalInput").ap()
    w_d = nc.dram_tensor("wpack", [108, 128, 128], dt.bfloat16, kind="ExternalInput").ap()
    b_d = nc.dram_tensor("bpack", [128, 6], dt.float32, kind="ExternalInput").ap()
    y_d = nc.dram_tensor("y", [C, H, W], dt.float32, kind="ExternalOutput").ap()

    with tile.TileContext(nc) as tc:
        from contextlib import ExitStack

        with ExitStack() as ctx:
            const = ctx.enter_context(tc.tile_pool(name="const", bufs=1))
            xstage = ctx.enter_context(tc.tile_pool(name="xstage", bufs=2))
            xpad_p = ctx.enter_context(tc.tile_pool(name="xpad", bufs=1))
            evac = ctx.enter_context(tc.tile_pool(name="evac", bufs=2))
            qload = ctx.enter_context(tc.tile_pool(name="qload", bufs=3))
            kload = ctx.enter_context(tc.tile_pool(name="kload", bufs=3))
            vload = ctx.enter_context(tc.tile_pool(name="vload", bufs=3))
            att = ctx.enter_context(tc.tile_pool(name="att", bufs=3))
            stat = ctx.enter_context(tc.tile_pool(name="stat", bufs=4))
            resid = ctx.enter_context(tc.tile_pool(name="resid", bufs=2))
            outp = ctx.enter_context(tc.tile_pool(name="outp", bufs=2))
            psum_c = ctx.enter_context(tc.tile_pool(name="psc", bufs=3, space="PSUM"))
            psum_s = ctx.enter_context(tc.tile_pool(name="pss", bufs=2, space="PSUM"))
            psum_y = ctx.enter_context(tc.tile_pool(name="psy", bufs=2, space="PSUM"))
            psum_z = ctx.enter_context(tc.tile_pool(name="psz", bufs=1, space="PSUM"))
            dram = ctx.enter_context(tc.tile_pool(name="dram", bufs=1, space="DRAM"))

            # ---- constants ----
            # weight order in w_d is q,k,v; the k slice is DMA'd first since
            # the k conv runs first
            w_sb = const.tile([128, 108, 128], dt.bfloat16)
            b_sb = const.tile([128, 6], dt.float32)
            nc.sync.dma_start(out=b_sb[:], in_=b_d)
            ones_f32 = const.tile([128, 128], dt.float32)
            nc.vector.memset(ones_f32[:], 1.0)

            # ---- padded bf16 x, natural [h, w] and transposed [w, h] ----
            xpad, xpadT = [], []
            for icc in range(2):
                t = xpad_p.tile([128, PAD, PAD], dt.bfloat16, tag=f"xpad{icc}")
                tt = xpad_p.tile([128, PAD, PAD], dt.bfloat16, tag=f"xpadT{icc}")
                for b in (t, tt):
                    nc.vector.memset(b[:, 0, :], 0.0)
                    nc.vector.memset(b[:, PAD - 1, :], 0.0)
                    nc.vector.memset(b[:, :, 0:1], 0.0)
                    nc.vector.memset(b[:, :, PAD - 1 : PAD], 0.0)
                xpad.append(t)
                xpadT.append(tt)
            xflat = x_d.rearrange("c h w -> c (h w)")
            # DVE casts each staged chunk into the natural layout (k/v convs
            # consume it incrementally); the transposed buffer is filled by
            # ScalarE copies FROM xpad, deferred into the k-conv chunk
            # stream so neither the staging slots nor the ACT queue stall.
            castT_jobs = []
            for s in range(16):
                for icc in range(2):
                    st = xstage.tile([128, 1024], dt.float32, tag="st")
                    nc.sync.dma_start(
                        out=st[:],
                        in_=xflat[icc * 128 : (icc + 1) * 128, s * 1024 : (s + 1) * 1024],
                    )
                    nc.vector.tensor_copy(
                        out=xpad[icc][:, 1 + s * 8 : 1 + s * 8 + 8, 1 : 1 + W],
                        in_=st.rearrange("p (h w) -> p h w", w=W),
                    )

                    def castT(s=s, icc=icc):
                        nc.scalar.activation(
                            out=xpadT[icc][:, 1 : 1 + W, 1 + s * 8 : 1 + s * 8 + 8],
                            in_=xpad[icc][
                                :, 1 + s * 8 : 1 + s * 8 + 8, 1 : 1 + W
                            ].rearrange("p h w -> p w h"),
                            func=AF.Copy,
                        )

                    castT_jobs.append(castT)
                if s == 0:
                    # k-conv weights right behind the first x chunks
                    nc.sync.dma_start(
                        out=w_sb[:, 36:72, :],
                        in_=w_d[36:72].rearrange("t p f -> p t f"),
                    )

            # remaining weights land while the k conv runs
            nc.sync.dma_start(
                out=w_sb[:, 72:108, :], in_=w_d[72:108].rearrange("t p f -> p t f")
            )
            nc.sync.dma_start(
                out=w_sb[:, 0:36, :], in_=w_d[0:36].rearrange("t p f -> p t f")
            )

            # ---- HBM round-trip buffers for q^T, k, v (bf16) ----
            # position-major [j, c, i] layout: the attention loads then read
            # one contiguous 2 KB run per partition (128 descriptors per
            # DMA instead of 1024), which keeps descriptor generation off
            # the critical path in the tail.
            qt_dram = dram.tile([128, C, 128], dt.bfloat16, tag="qt")
            k_dram = dram.tile([128, C, 128], dt.bfloat16, tag="kd")
            v_dram = dram.tile([128, C, 128], dt.bfloat16, tag="vd")
            cv_dram = [qt_dram, k_dram, v_dram]

            def conv_chunk(occ, cv, ch):
                colmajor = cv == 0
                ps = psum_c.tile([128, 512], dt.float32, tag="psc")
                for i, (icc, kk) in enumerate(
                    [(a, b) for a in range(2) for b in range(9)]
                ):
                    dy, dx = kk // 3, kk % 3
                    if colmajor:
                        rhs = xpadT[icc][
                            :, 4 * ch + dx : 4 * ch + dx + 4, dy : dy + H
                        ]
                    else:
                        rhs = xpad[icc][
                            :, 4 * ch + dy : 4 * ch + dy + 4, dx : dx + W
                        ]
                    nc.tensor.matmul(
                        ps[:],
                        lhsT=w_sb[:, cv * 36 + kk * 4 + icc * 2 + occ, :],
                        rhs=rhs,
                        start=(i == 0),
                        stop=(i == 17),
                    )
                ev = evac.tile([128, 512], dt.bfloat16, tag="ev")
                nc.scalar.activation(
                    out=ev[:],
                    in_=ps[:],
                    func=AF.Relu if cv < 2 else AF.Identity,
                    bias=b_sb[:, cv * 2 + occ : cv * 2 + occ + 1],
                    scale=1.0,
                )
                nc.sync.dma_start(
                    out=cv_dram[cv][
                        4 * ch : 4 * ch + 4, occ * 128 : (occ + 1) * 128, :
                    ].rearrange("j c i -> c j i"),
                    in_=ev.rearrange("c (j i) -> c j i", i=128),
                )

            # Attention runs in pairs of 4-channel waves: one batched DMA
            # set per 8 channels (halves Sync-queue descriptor-gen cost,
            # which otherwise paces the tail), stores on the idle GpSimd
            # queue. Per wave: 4 S-matmuls into one PSUM bank, ONE batched
            # exp on ScalarE, one DVE reduce for the per-channel sums, the
            # global-sum broadcast matmul, 1/Z scale of P, 4 y-matmuls,
            # fp32 residual add.
            def att_load(occ, g0):
                c0 = occ * 128 + g0
                qt8 = qload.tile([128, 8, 128], dt.bfloat16, tag="qt8")
                nc.sync.dma_start(out=qt8[:], in_=qt_dram[:, c0 : c0 + 8, :])
                k8 = kload.tile([128, 8, 128], dt.bfloat16, tag="k8")
                nc.sync.dma_start(out=k8[:], in_=k_dram[:, c0 : c0 + 8, :])
                v8 = vload.tile([128, 8, 128], dt.bfloat16, tag="v8")
                nc.sync.dma_start(out=v8[:], in_=v_dram[:, c0 : c0 + 8, :])
                xr8 = resid.tile([128, 8, 128], dt.float32, tag="xr8")
                nc.sync.dma_start(
                    out=xr8[:], in_=x_d[c0 : c0 + 8].rearrange("c h w -> h c w")
                )
                out8 = outp.tile([128, 8, 128], dt.float32, tag="out8")
                return qt8, k8, v8, xr8, out8

            def att_wave(tiles, w):
                qt8, k8, v8, xr8, out8 = tiles
                o = 4 * w
                ps_s = psum_s.tile([128, 4, 128], dt.float32, tag="pss")
                for j in range(4):
                    nc.tensor.matmul(
                        ps_s[:, j, :],
                        lhsT=k8[:, o + j, :],
                        rhs=qt8[:, o + j, :],
                        start=True,
                        stop=True,
                    )
                p4 = att.tile([128, 4, 128], dt.bfloat16, tag="p4")
                nc.scalar.activation(out=p4[:], in_=ps_s[:], func=AF.Exp)
                cs4 = stat.tile([128, 4], dt.float32, tag="cs4")
                nc.vector.reduce_sum(cs4[:], p4[:], axis=mybir.AxisListType.X)
                ps_z = psum_z.tile([128, 4], dt.float32, tag="psz")
                nc.tensor.matmul(
                    ps_z[:], lhsT=ones_f32[:], rhs=cs4[:], start=True, stop=True
                )
                rec4 = stat.tile([128, 4], dt.float32, tag="rec4")
                nc.vector.reciprocal(rec4[:], ps_z[:])
                ps_y = psum_y.tile([128, 4, 128], dt.float32, tag="psy")
                for j in range(4):
                    nc.tensor.matmul(
                        ps_y[:, j, :],
                        lhsT=p4[:, j, :],
                        rhs=v8[:, o + j, :],
                        start=True,
                        stop=True,
                    )
                # out = (P~ @ v) / Z + x in one fused DVE op per channel
                for j in range(4):
                    nc.vector.scalar_tensor_tensor(
                        out=out8[:, o + j, :],
                        in0=ps_y[:, j, :],
                        scalar=rec4[:, j : j + 1],
                        in1=xr8[:, o + j, :],
                        op0=mybir.AluOpType.mult,
                        op1=mybir.AluOpType.add,
                    )

            def att_store(occ, g0, tiles):
                c0 = occ * 128 + g0
                nc.gpsimd.dma_start(
                    out=y_d[c0 : c0 + 8].rearrange("c h w -> h c w"), in_=tiles[4][:]
                )

            def att_steps(occ):
                for g0 in range(0, 128, 8):
                    tiles = []

                    def load(g0=g0, tiles=tiles):
                        tiles.append(att_load(occ, g0))

                    def wave0(tiles=tiles):
                        att_wave(tiles[0], 0)

                    def wave1(g0=g0, tiles=tiles):
                        att_wave(tiles[0], 1)
                        att_store(occ, g0, tiles[0])

                    yield load
                    yield wave0
                    yield wave1

            # Phase A: block-0 convs (k, v first; q last so the xpadT fill
            # hides under the k/v matmuls). One deferred transposed-cast
            # per early chunk keeps ScalarE ahead of the PSUM evacuations.
            for i, (cv, ch) in enumerate(
                [(a, b) for a in (1, 2, 0) for b in range(32)]
            ):
                conv_chunk(0, cv, ch)
                if i % 2 == 0 and i // 2 < len(castT_jobs):
                    castT_jobs[i // 2]()
            # Phase B: block-1 convs with block-0 attention steps woven in
            # (one step per 2 conv chunks keeps TensorE dense while the
            # attention DMA/ACT/DVE chain rides along).
            steps = att_steps(0)
            for i, (cv, ch) in enumerate(
                [(a, b) for a in (1, 2, 0) for b in range(32)]
            ):
                conv_chunk(1, cv, ch)
                if i % 2 == 1:
                    step = next(steps, None)
                    if step is not None:
                        step()
            for step in steps:
                step()
            # Phase C: block-1 attention tail.
            for step in att_steps(1):
                step()

    nc.compile()
    return nc


def _get_program():
    global _PROG
    if _PROG is None:
        _PROG = _build_program()
    return _PROG


def _pack_weights(Wq, Wk, Wv):
    packs = []
    for Wcv in (Wq, Wk, Wv):
        a = np.asarray(Wcv, np.float32).transpose(2, 3, 1, 0)  # [dy, dx, i, o]
        a = a.reshape(9, 2, 128, 2, 128)  # [kk, icc, i, occ, o]
        a = a.transpose(0, 1, 3, 2, 4)  # [kk, icc, occ, i, o]
        packs.append(a.reshape(36, 128, 128))
    return np.concatenate(packs, axis=0).astype(ml_dtypes.bfloat16)


def _run(inputs, trace=False, trace_kwargs=None):
    from concourse.bass_utils import run_bass_kernel_spmd

    nc = _get_program()
    x = np.ascontiguousarray(np.asarray(inputs["x"], np.float32))
    wpack = _pack_weights(inputs["Wq"], inputs["Wk"], inputs["Wv"])
    bq = np.asarray(inputs["bq"], np.float32)
    bk = np.asarray(inputs["bk"], np.float32)
    bv = np.asarray(inputs["bv"], np.float32)
    bpack = np.stack(
        [bq[:128], bq[128:], bk[:128], bk[128:], bv[:128], bv[128:]], axis=1
    )
    bpack = np.ascontiguousarray(bpack, dtype=np.float32)  # [128, 6]

    in_maps = [
        {"x": x[b], "wpack": wpack, "bpack": bpack} for b in range(N_CORES)
    ]
    res = run_bass_kernel_spmd(
        nc,
        in_maps,
        core_ids=list(range(N_CORES)),
        trace=trace,
        **(trace_kwargs or {}),
    )
    out = np.stack([res.results[b]["y"] for b in range(N_CORES)], axis=0)
    return out, res


def kernel(**inputs) -> np.ndarray:
    out, _ = _run(inputs, trace=False)
    return out


def kernel_traced(inputs):
    try:
        import axon_shim

        axon_shim.install()
    except Exception:
        pass
    out, res = _run(inputs, trace=True)
    return out, res
